# revision 1
# baseline (speedup 1.0000x reference)
"""MoE gate (top-6 routing) Trainium2 Bass kernel.

Problem: hidden_states [4, 4096, 2048] f32, gate weight [64, 2048] f32.
  logits = x @ W.T            -> [16384, 64]
  topk_weight, topk_idx = top_k(logits, 6)
  topk_weight = softmax(topk_weight)   (the reference's extra
  normalization divides by 1.0 + 1e-20 and is a no-op in fp32)
Returns (topk_idx int32 [16384, 6], topk_weight f32 [16384, 6]).

Sharding: data-parallel over tokens. Each of the 8 cores gets 2048
tokens; the gate weight is replicated.

Precision scheme (fp32-accurate at half the HBM traffic): each fp32
value is split on the host into two fp16 halves,
    xh = fp16(x),  xl = fp16((x - xh) * 2^11)
so x = xh + 2^-11*xl to ~2^-23 relative precision. Then
    logits = xh@wh.T + 2^-11 * (xh@wl.T + xl@wh.T)      (+O(2^-22) term dropped)
which matches the fp32 reference to below fp32 accumulation noise
(verified: bit-level top-6 agreement with the jax fp32 reference on the
actual test inputs). fp16 matmuls stream at 1 cycle/row (vs 4 for
fp32) and the input stream is 8 MB/core instead of 16.

Per-core kernel:
  - x halves fed pre-transposed ([H, T] layout, fp16) so the
    contraction dim lands on SBUF partitions with contiguous DMAs;
    w halves fed pre-packed as [128, 16*64] fp16
  - two 1024-token super-panels streamed panel-major (panel 0's top-k
    overlaps panel 1's DMA); x loaded in 1 MiB chunks of 4 h-tiles
  - matmuls in [E, T'] orientation (w stationary, x moving at N=512),
    2-way column-tiled: a panel's two 512-token blocks accumulate
    concurrently in partition halves [0:64]/[64:128] of PSUM banks
  - combine P1 + 2^-11*P2 (ACT scaled copy + DVE add)
  - PE-transpose of the logits to [token, expert] tiles
  - DVE max8/max_index (from PSUM) -> top-8 values + indices
  - ACT exp(v - max) with accumulated sum, DVE reciprocal + scale
  - results staged in SBUF, per-half-panel DMAs out; host de-interleaves
"""

import numpy as np

import concourse.mybir as mybir
import concourse.tile as tile
from concourse import bacc
from concourse.bass_utils import run_bass_kernel_spmd

f32 = mybir.dt.float32
f16 = mybir.dt.float16
u32 = mybir.dt.uint32
i32 = mybir.dt.int32

N_CORES = 8
B, S, H = 4, 4096, 2048
E = 64
TOP_K = 6
T_FULL = B * S              # 16384 tokens
T_CORE = T_FULL // N_CORES  # 2048 tokens per core
KT = H // 128               # 16 contraction tiles
NTT = T_CORE // 128         # 16 token tiles per core
TB = 512                    # tokens per matmul block (PSUM bank = 512 fp32)
PANEL = 2 * TB              # 1024 tokens per super-panel (one packed psum pair)
NP = T_CORE // PANEL        # 2 super-panels per core
# h-tiles per DMA chunk, per panel: small first chunks so the PE can
# start ~5us earlier; 1 MiB steady-state chunks after.
CHUNKS = [
    [1, 1, 2, 4, 8],         # panel 0: small first chunks (early PE start)
    [2, 4, 4, 4, 1, 1],      # panel 1: early start AND small last chunks
]
LSCALE = float(2.0 ** -11)

_CACHE = {}


def _build():
    nc = bacc.Bacc("TRN2", target_bir_lowering=False, debug=False)
    # x halves host-packed per DMA chunk: flat [128, KT*NP*PANEL]; chunk
    # (q, c) of sz h-tiles occupies columns [off*PANEL : (off+sz)*PANEL) where
    # off is the running h-tile offset in stream order.
    XCOLS = KT * NP * PANEL
    xh = nc.dram_tensor("xh", [128, XCOLS], f16, kind="ExternalInput").ap()
    xl = nc.dram_tensor("xl", [128, XCOLS], f16, kind="ExternalInput").ap()
    wh = nc.dram_tensor("wh", [128, KT * E], f16, kind="ExternalInput").ap()
    wl = nc.dram_tensor("wl", [128, KT * E], f16, kind="ExternalInput").ap()
    ident = nc.dram_tensor("ident", [E, E], f32, kind="ExternalInput").ap()
    out_w = nc.dram_tensor("out_w", [128, NTT * TOP_K], f32, kind="ExternalOutput").ap()
    out_i = nc.dram_tensor("out_i", [128, NTT * 8], i32, kind="ExternalOutput").ap()

    with tile.TileContext(nc) as tc:
        with (
            tc.tile_pool(name="persist", bufs=1) as persist,
            tc.tile_pool(name="work", bufs=4) as work,
            tc.tile_pool(name="psum", bufs=2, space="PSUM") as psp,
            tc.tile_pool(name="psumT", bufs=4, space="PSUM") as pspT,
        ):
            # ---- weights first (warmups depend on them), then x chunks ----
            wh_all = persist.tile([128, KT * E], f16, tag="wh_all")
            nc.sync.dma_start(out=wh_all, in_=wh)
            wl_all = persist.tile([128, KT * E], f16, tag="wl_all")
            nc.sync.dma_start(out=wl_all, in_=wl)

            # per (q, h-tile a) -> (sbuf tile, col offset within tile)
            xh_at = {}
            xl_at = {}
            _off = 0

            def load_chunk(q, c, sz, off):
                th = persist.tile([128, sz * PANEL], f16, tag=f"xh{q}_{c}")
                nc.sync.dma_start(
                    out=th, in_=xh[:, off * PANEL : (off + sz) * PANEL]
                )
                tl = persist.tile([128, sz * PANEL], f16, tag=f"xl{q}_{c}")
                nc.sync.dma_start(
                    out=tl, in_=xl[:, off * PANEL : (off + sz) * PANEL]
                )
                a0 = sum(CHUNKS[q][:c])
                for j in range(sz):
                    xh_at[(q, a0 + j)] = (th, j)
                    xl_at[(q, a0 + j)] = (tl, j)

            for c, sz in enumerate(CHUNKS[0]):
                load_chunk(0, c, sz, _off)
                _off += sz
            id_t = persist.tile([E, E], f32, tag="ident")
            nc.sync.dma_start(out=id_t, in_=ident)
            for c, sz in enumerate(CHUNKS[1]):
                load_chunk(1, c, sz, _off)
                _off += sz

            # Warmup matmuls: absorb the wh/wl DMA waits on the PE (a fused
            # matmul carries at most one semaphore wait) and spin the PE so
            # the HAM clock-gate warms before the real matmuls arrive.
            ps_warm = pspT.tile([64, 64], f32, tag="ps_t")
            for _ in range(6):
                nc.tensor.matmul(
                    ps_warm, wh_all[:, 0:64], wh_all[:, 0:64], start=True, stop=True
                )
            nc.tensor.matmul(
                ps_warm, wl_all[:, 0:64], wl_all[:, 0:64], start=True, stop=True
            )
            # absorb the ident DMA wait + warm the transpose path
            nc.tensor.transpose(ps_warm, id_t, id_t)

            stage_w = persist.tile([128, NTT * TOP_K], f32, tag="stage_w")
            stage_i = persist.tile([128, NTT * 8], u32, tag="stage_i")

            for q in range(NP):
                # ---- packed accumulation; half -> partition range / col-group
                ps1 = psp.tile([128, TB], f32, tag="ps1")  # xh@wh
                ps2 = psp.tile([128, TB], f32, tag="ps2")  # xh@wl + xl@wh
                def mm_p1_p2a(a):
                    wh_t = wh_all[:, a * E : (a + 1) * E]
                    wl_t = wl_all[:, a * E : (a + 1) * E]
                    th, jh = xh_at[(q, a)]
                    for half in range(2):
                        slh = slice(jh * PANEL + half * TB, jh * PANEL + (half + 1) * TB)
                        pr = slice(half * 64, (half + 1) * 64)
                        nc.tensor.matmul(
                            ps1[pr, :], wh_t, th[:, slh],
                            start=(a == 0), stop=(a == KT - 1),
                        )
                        nc.tensor.matmul(
                            ps2[pr, :], wl_t, th[:, slh],
                            start=(a == 0), stop=False,
                        )

                def mm_p2b(a):
                    wh_t = wh_all[:, a * E : (a + 1) * E]
                    tl, jl = xl_at[(q, a)]
                    for half in range(2):
                        sll = slice(jl * PANEL + half * TB, jl * PANEL + (half + 1) * TB)
                        pr = slice(half * 64, (half + 1) * 64)
                        nc.tensor.matmul(
                            ps2[pr, :], wh_t, tl[:, sll],
                            start=False, stop=(a == KT - 1),
                        )

                # xh-consuming matmuls run as chunks land; the xl-consuming
                # ones trail one chunk behind so they never stall the PE FIFO
                # (the xl chunk lands while the next xh chunk's work runs).
                bounds = []
                a0 = 0
                for sz in CHUNKS[q]:
                    bounds.append((a0, a0 + sz))
                    a0 += sz
                for ci, (lo, hi) in enumerate(bounds):
                    for a in range(lo, hi):
                        mm_p1_p2a(a)
                    if ci > 0:
                        plo, phi = bounds[ci - 1]
                        for a in range(plo, phi):
                            mm_p2b(a)
                lo, hi = bounds[-1]
                for a in range(lo, hi):
                    mm_p2b(a)

                # ---- per-block epilogue (per 128-token tile for a short
                #      tail: combine -> transpose -> top-k pipeline) ----
                for half in range(2):
                    pr = slice(half * 64, (half + 1) * 64)
                    lt_half = {}
                    for cc in range(TB // 256):
                        # combine at 256-col granularity: amortizes the fixed
                        # ACT/DVE op costs 2x vs per-tile, keeps lead-in short
                        cs2 = slice(cc * 256, (cc + 1) * 256)
                        t2 = work.tile([64, 256], f32, tag="t2")
                        nc.scalar.activation(
                            out=t2,
                            in_=ps2[pr, cs2],
                            func=mybir.ActivationFunctionType.Copy,
                            scale=LSCALE,
                        )
                        ltE = work.tile([64, 256], f32, tag="ltE")
                        nc.vector.tensor_add(ltE, t2, ps1[pr, cs2])
                        lt_half[cc] = ltE
                    for tt in range(TB // 128):
                        t = (2 * q + half) * (TB // 128) + tt
                        ltE = lt_half[tt // 2]
                        cs = slice((tt % 2) * 128, (tt % 2 + 1) * 128)

                        ps_t = pspT.tile([128, E], f32, tag="ps_t")
                        nc.tensor.transpose(ps_t, ltE[:, cs], id_t)
                        m8 = work.tile([128, 8], f32, tag="m8")
                        nc.vector.max(out=m8, in_=ps_t)
                        nc.vector.max_index(
                            stage_i[:, t * 8 : (t + 1) * 8], m8, ps_t
                        )

                        negm = work.tile([128, 1], f32, tag="negm")
                        nc.scalar.mul(negm, m8[:, 0:1], -1.0)
                        expw = work.tile([128, TOP_K], f32, tag="expw")
                        ssum = work.tile([128, 1], f32, tag="ssum")
                        nc.scalar.activation(
                            out=expw,
                            in_=m8[:, 0:TOP_K],
                            func=mybir.ActivationFunctionType.Exp,
                            bias=negm[:, 0:1],
                            scale=1.0,
                            accum_out=ssum[:, 0:1],
                        )
                        rsum = work.tile([128, 1], f32, tag="rsum")
                        nc.vector.reciprocal(rsum, ssum)
                        nc.vector.tensor_scalar_mul(
                            stage_w[:, t * TOP_K : (t + 1) * TOP_K],
                            expw,
                            rsum[:, 0:1],
                        )

            # ---- output DMAs, emitted last so their chain-waits can never
            #      head-of-line-block the x load triggers on the Sync ring ----
            for q in range(NP):
                for half in range(2):
                    nt_h = TB // 128  # 4 token tiles per half
                    c0 = (2 * q + half) * nt_h
                    nc.sync.dma_start(
                        out=out_w[:, c0 * TOP_K : (c0 + nt_h) * TOP_K],
                        in_=stage_w[:, c0 * TOP_K : (c0 + nt_h) * TOP_K],
                    )
                    nc.sync.dma_start(
                        out=out_i[:, c0 * 8 : (c0 + nt_h) * 8],
                        in_=stage_i[:, c0 * 8 : (c0 + nt_h) * 8].bitcast(i32),
                    )

    nc.compile()
    return nc


def _get_nc():
    if "nc" not in _CACHE:
        _CACHE["nc"] = _build()
    return _CACHE["nc"]


def _split_fp16(arr32):
    """arr32 (fp32) -> (hi fp16, lo fp16) with arr32 ~= hi + 2^-11 * lo."""
    hi = arr32.astype(np.float16)
    lo = ((arr32 - hi.astype(np.float32)) * 2048.0).astype(np.float16)
    return hi, lo


def kernel(hidden_states: np.ndarray, weight: np.ndarray, **_run_kwargs):
    x = np.ascontiguousarray(hidden_states, dtype=np.float32).reshape(T_FULL, H)
    w = np.ascontiguousarray(weight, dtype=np.float32)

    w_hi, w_lo = _split_fp16(w)  # [E, H] fp16
    # device layout [128, KT*E]: row p, col a*E+e  <-  W[e, a*128+p]
    def pack_w(wx):
        return np.ascontiguousarray(
            wx.T.reshape(KT, 128, E).transpose(1, 0, 2).reshape(128, KT * E)
        )

    whp = pack_w(w_hi)
    wlp = pack_w(w_lo)
    ident = np.eye(E, dtype=np.float32)

    def pack_x(xT16):
        # [H, T_CORE] -> [128, KT*NP*PANEL] in stream order: for panel q and
        # h-tile a (ascending), column block (q, a) = xT16[a*128+p, q*PANEL+t]
        v = xT16.reshape(KT, 128, NP, PANEL)
        return np.ascontiguousarray(
            v.transpose(1, 2, 0, 3).reshape(128, NP * KT * PANEL)
        )

    in_maps = []
    for c in range(N_CORES):
        shard = x[c * T_CORE : (c + 1) * T_CORE, :]  # [T_CORE, H]
        xT = np.ascontiguousarray(shard.T)  # [H, T_CORE] fp32
        xhs, xls = _split_fp16(xT)
        in_maps.append(
            {"xh": pack_x(xhs), "xl": pack_x(xls), "wh": whp, "wl": wlp, "ident": ident}
        )

    nc = _get_nc()
    res = run_bass_kernel_spmd(
        nc, in_maps, core_ids=list(range(N_CORES)), **_run_kwargs
    )

    idx_parts = []
    w_parts = []
    for c in range(N_CORES):
        r = res.results[c]
        si = r["out_i"].reshape(128, NTT, 8).transpose(1, 0, 2)[:, :, :TOP_K]
        sw = r["out_w"].reshape(128, NTT, TOP_K).transpose(1, 0, 2)
        idx_parts.append(si.reshape(T_CORE, TOP_K).astype(np.int32, copy=False))
        w_parts.append(sw.reshape(T_CORE, TOP_K))

    topk_idx = np.concatenate(idx_parts, axis=0)
    topk_weight = np.concatenate(w_parts, axis=0)
    if "trace" in _run_kwargs:
        return (topk_idx, topk_weight), res
    return topk_idx, topk_weight



# revision 2
# speedup vs baseline: 1.0246x; 1.0246x over previous
"""MoE gate (top-6 routing) Trainium2 Bass kernel, v2.

Problem: hidden_states [4, 4096, 2048] f32, gate weight [64, 2048] f32.
  logits = x @ W.T            -> [16384, 64]
  topk_weight, topk_idx = top_k(logits, 6)
  topk_weight = softmax(topk_weight)
Returns (topk_idx int32 [16384, 6], topk_weight f32 [16384, 6]).

Sharding: data-parallel over tokens; 2048 tokens/core, weight replicated.

Precision (identical math to the verified baseline): x and w are split on
the host into fp16 halves, v = vh + 2^-11*vl, giving ~2^-22 relative
precision. logits = xh@wh + 2^-11*(xh@wl + xl@wh), bit-level top-6
agreement with the fp32 reference on the test inputs.

v2 structure (changes vs baseline, driven by the baseline's trace):
  - Stationary operand packs [wh | wl] 128 wide: ONE ldweights per k-tile
    feeds two N=512 matmuls (xh -> psA has main logits in partitions 0:64
    and the w-correction xh@wl in 64:128; xl -> psB has xl@wh in 0:64).
    128 matmuls total (vs 192) and full-width FWL weight loads.
  - Block-major streaming: 4 blocks of 512 tokens; per block the 16
    k-tiles arrive xh/xl-interleaved in 4x 1 MiB DMA chunks. Each block's
    top-k epilogue overlaps the next block's DMA stream instead of
    serializing at the end (the baseline lost ~20 us in its tail).
  - HAM warmup: a junk-matmul burst on a memset tile right at kernel
    start keeps the PE busy ~4 us so the clock gate opens (2.4 GHz)
    before the real matmuls; the baseline ran at 1.2 GHz for 34 us.
  - Per-block epilogue: combine psA/psB (ACT copy-scale + DVE
    scalar_tensor_tensor + add), PE-transpose 128-token tiles to
    [token, expert], DVE max8/max_index, ACT exp(accum), DVE
    reciprocal+scale. Weights and indices staged in one u32 tile; one
    output DMA per block.
"""

import numpy as np

import concourse.mybir as mybir
import concourse.tile as tile
from concourse import bacc
from concourse.bass_utils import run_bass_kernel_spmd

f32 = mybir.dt.float32
f16 = mybir.dt.float16
u32 = mybir.dt.uint32
i32 = mybir.dt.int32

N_CORES = 8
B, S, H = 4, 4096, 2048
E = 64
TOP_K = 6
T_FULL = B * S              # 16384 tokens
T_CORE = T_FULL // N_CORES  # 2048 tokens per core
KT = H // 128               # 16 contraction tiles
NB = 4                      # token blocks per core
TB = T_CORE // NB           # 512 tokens per block (one PSUM bank)
NTT = T_CORE // 128         # 16 token tiles per core
TPB = TB // 128             # 4 token tiles per block
KG = 4                      # k-tiles per DMA chunk (1 MiB chunks)
NCH = NB * (KT // KG)       # 16 chunks per core
OC = TOP_K + 8              # 14 staged u32 cols per token tile (6 w + 8 idx)
LSCALE = float(2.0 ** -11)

_CACHE = {}


def _build():
    nc = bacc.Bacc("TRN2", target_bir_lowering=False, debug=False)
    # x stream layout [128, NB*KT*1024] fp16: col b*(KT*1024) + a*1024 +
    # half*512 + t, holding xh (half=0) / xl (half=1) for block b, k-tile a.
    x = nc.dram_tensor("x", [128, NB * KT * 1024], f16, kind="ExternalInput").ap()
    # stationary pack [128, KT*128]: cols a*128+j = wh[j] (j<64) | wl[j-64]
    w2 = nc.dram_tensor("w2", [128, KT * 128], f16, kind="ExternalInput").ap()
    ident = nc.dram_tensor("ident", [E, E], f32, kind="ExternalInput").ap()
    out = nc.dram_tensor("out", [128, NTT * OC], u32, kind="ExternalOutput").ap()

    with tile.TileContext(nc) as tc:
        with (
            tc.tile_pool(name="persist", bufs=1) as persist,
            tc.tile_pool(name="work", bufs=4) as work,
            tc.tile_pool(name="psA", bufs=2, space="PSUM") as psAp,
            tc.tile_pool(name="psB", bufs=2, space="PSUM") as psBp,
            tc.tile_pool(name="psT", bufs=2, space="PSUM") as psTp,
            tc.tile_pool(name="psW", bufs=1, space="PSUM") as psWp,
        ):
            # ---- input DMAs, in stream order ----
            w2_t = persist.tile([128, KT * 128], f16, tag="w2")
            nc.sync.dma_start(out=w2_t, in_=w2)
            id_t = persist.tile([E, E], f32, tag="ident")
            nc.sync.dma_start(out=id_t, in_=ident)
            xch = []
            for c in range(NCH):
                t = persist.tile([128, KG * 1024], f16, tag=f"x{c}")
                nc.sync.dma_start(out=t, in_=x[:, c * KG * 1024 : (c + 1) * KG * 1024])
                xch.append(t)

            stage = persist.tile([128, NTT * OC], u32, tag="stage")

            # ---- HAM warmup: junk matmuls on a memset tile so the PE
            #      clock-gate opens before the real matmuls arrive ----
            wm = persist.tile([128, 512], f16, tag="warm")
            nc.gpsimd.memset(wm, 0.25)
            wps = psWp.tile([128, 512], f32, tag="wps")
            for _ in range(9):
                nc.tensor.matmul(wps, wm[:, 0:128], wm, start=True, stop=True)

            def emit_block_mms(b):
                psA = psAp.tile([128, TB], f32, tag="psA")
                psB = psBp.tile([128, TB], f32, tag="psB")
                for g in range(KT // KG):
                    ch = xch[b * (KT // KG) + g]
                    for j in range(KG):
                        a = g * KG + j
                        lhs = w2_t[:, a * 128 : (a + 1) * 128]
                        nc.tensor.matmul(
                            psA, lhs, ch[:, j * 1024 : j * 1024 + 512],
                            start=(a == 0), stop=(a == KT - 1),
                        )
                        nc.tensor.matmul(
                            psB, lhs, ch[:, j * 1024 + 512 : (j + 1) * 1024],
                            start=(a == 0), stop=(a == KT - 1),
                        )
                    yield g, psA, psB

            def emit_combine(psA, psB):
                # lt[0:64] = psA[0:64] + 2^-11*(psA[64:128] + psB[0:64])
                s1 = work.tile([E, TB], f32, tag="s1")
                nc.scalar.activation(
                    out=s1, in_=psB[0:E, :],
                    func=mybir.ActivationFunctionType.Copy, scale=LSCALE,
                )
                t2 = work.tile([E, TB], f32, tag="t2")
                nc.vector.scalar_tensor_tensor(
                    t2, psA[E:128, :], LSCALE, s1,
                    mybir.AluOpType.mult, mybir.AluOpType.add,
                )
                lt = work.tile([E, TB], f32, tag="lt")
                nc.vector.tensor_add(lt, t2, psA[0:E, :])
                return lt

            def emit_topk(b, lt):
                for tt in range(TPB):
                    t = b * TPB + tt
                    ps_t = psTp.tile([128, E], f32, tag="ps_t")
                    nc.tensor.transpose(ps_t, lt[:, tt * 128 : (tt + 1) * 128], id_t)
                    m8 = work.tile([128, 8], f32, tag="m8")
                    nc.vector.max(out=m8, in_=ps_t)
                    nc.vector.max_index(
                        stage[:, t * OC + TOP_K : (t + 1) * OC], m8, ps_t
                    )
                    negm = work.tile([128, 1], f32, tag="negm")
                    nc.scalar.mul(negm, m8[:, 0:1], -1.0)
                    expw = work.tile([128, TOP_K], f32, tag="expw")
                    ssum = work.tile([128, 1], f32, tag="ssum")
                    nc.scalar.activation(
                        out=expw,
                        in_=m8[:, 0:TOP_K],
                        func=mybir.ActivationFunctionType.Exp,
                        bias=negm[:, 0:1],
                        scale=1.0,
                        accum_out=ssum[:, 0:1],
                    )
                    rsum = work.tile([128, 1], f32, tag="rsum")
                    nc.vector.reciprocal(rsum, ssum)
                    nc.vector.tensor_scalar_mul(
                        stage[:, t * OC : t * OC + TOP_K].bitcast(f32),
                        expw,
                        rsum[:, 0:1],
                    )
                nc.sync.dma_start(
                    out=out[:, b * TPB * OC : (b + 1) * TPB * OC],
                    in_=stage[:, b * TPB * OC : (b + 1) * TPB * OC],
                )

            pending = None  # (b, lt) whose transposes/top-k are deferred
            for b in range(NB):
                for g, psA, psB in emit_block_mms(b):
                    # previous block's PE transposes + top-k go after this
                    # block's second chunk of matmuls: their ACT/DVE combine
                    # inputs are ready by then, so the PE never stalls on
                    # them at the block boundary.
                    if g == 1 and pending is not None:
                        emit_topk(*pending)
                        pending = None
                lt = emit_combine(psA, psB)
                pending = (b, lt)
            emit_topk(*pending)

    nc.compile()
    return nc


def _get_nc():
    if "nc" not in _CACHE:
        _CACHE["nc"] = _build()
    return _CACHE["nc"]


def _split_fp16(arr32):
    """arr32 (fp32) -> (hi fp16, lo fp16) with arr32 ~= hi + 2^-11 * lo."""
    hi = arr32.astype(np.float16)
    lo = ((arr32 - hi.astype(np.float32)) * 2048.0).astype(np.float16)
    return hi, lo


def kernel(hidden_states: np.ndarray, weight: np.ndarray, **_run_kwargs):
    x = np.ascontiguousarray(hidden_states, dtype=np.float32).reshape(T_FULL, H)
    w = np.ascontiguousarray(weight, dtype=np.float32)

    w_hi, w_lo = _split_fp16(w)  # [E, H] fp16
    # [128, KT*128]: row p, col a*128+j  <-  (wh|wl)[j, a*128+p]
    wh_r = np.ascontiguousarray(w_hi.T).reshape(KT, 128, E)
    wl_r = np.ascontiguousarray(w_lo.T).reshape(KT, 128, E)
    w2p = np.ascontiguousarray(
        np.concatenate([wh_r, wl_r], axis=2).transpose(1, 0, 2).reshape(128, KT * 128)
    )
    ident = np.eye(E, dtype=np.float32)

    def pack_x(xT16_h, xT16_l):
        # [H, T_CORE] halves -> [128, NB*KT*1024] stream order
        vh = xT16_h.reshape(KT, 128, NB, TB)  # [a, p, b, t]
        vl = xT16_l.reshape(KT, 128, NB, TB)
        X = np.empty((128, NB, KT, 2, TB), dtype=np.float16)
        X[:, :, :, 0, :] = vh.transpose(1, 2, 0, 3)
        X[:, :, :, 1, :] = vl.transpose(1, 2, 0, 3)
        return np.ascontiguousarray(X.reshape(128, NB * KT * 1024))

    in_maps = []
    for c in range(N_CORES):
        shard = x[c * T_CORE : (c + 1) * T_CORE, :]  # [T_CORE, H]
        xT = np.ascontiguousarray(shard.T)  # [H, T_CORE] fp32
        xhs, xls = _split_fp16(xT)
        in_maps.append({"x": pack_x(xhs, xls), "w2": w2p, "ident": ident})

    nc = _get_nc()
    res = run_bass_kernel_spmd(
        nc, in_maps, core_ids=list(range(N_CORES)), **_run_kwargs
    )

    idx_parts = []
    w_parts = []
    for c in range(N_CORES):
        r = res.results[c]
        v = r["out"].reshape(128, NTT, OC).transpose(1, 0, 2)  # [tile, tok, col]
        idx = np.ascontiguousarray(v[:, :, TOP_K : TOP_K + TOP_K])
        wts = np.ascontiguousarray(v[:, :, 0:TOP_K]).view(np.uint32)
        idx_parts.append(
            idx.reshape(T_CORE, TOP_K).astype(np.int32, copy=False)
        )
        w_parts.append(wts.view(np.float32).reshape(T_CORE, TOP_K))

    topk_idx = np.concatenate(idx_parts, axis=0)
    topk_weight = np.concatenate(w_parts, axis=0)
    if "trace" in _run_kwargs:
        return (topk_idx, topk_weight), res
    return topk_idx, topk_weight


# revision 4
# speedup vs baseline: 1.0408x; 1.0159x over previous
"""MoE gate (top-6 routing) Trainium2 Bass kernel, v3.

Problem: hidden_states [4, 4096, 2048] f32, gate weight [64, 2048] f32.
  logits = x @ W.T            -> [16384, 64]
  topk_weight, topk_idx = top_k(logits, 6)
  topk_weight = softmax(topk_weight)
Returns (topk_idx int32 [16384, 6], topk_weight f32 [16384, 6]).

Sharding: data-parallel over tokens; 2048 tokens/core, weight replicated.

Precision (identical math to the verified baseline): x and w are split on
the host into fp16 halves, v = vh + 2^-11*vl, giving ~2^-22 relative
precision. logits = xh@wh + 2^-11*(xh@wl + xl@wh), bit-level top-6
agreement with the fp32 reference on the test inputs.

v3 structure (evidence-driven, from the v1/v2 traces):
  - Column-group concurrency: each 512-token block is split into two
    256-token halves assigned to PE column groups 0/1 (psum partitions
    0:64 / 64:128). Two matmuls in different column groups stream
    concurrently (~114 ns/MM measured for this pattern vs 379 ns for
    full-width), so the PE tracks the DMA stream instead of over-running
    it by 13 us. Stationaries are the 64-wide wh / wl k-tiles.
  - Block-major streaming: per block b and k-tile a the stream holds
    [xh | xl] (1024 cols); 6 N=256 matmuls per (b, a): xh@wh -> psM,
    xh@wl -> psC, xl@wh -> psC, each on both halves. psM/psC accumulate
    over the 16 k-tiles; per-block epilogue overlaps the next block's
    stream.
  - HAM warmup: 16 junk matmuls (~7 us cold) guarantee a full busy
    window so the PE clock-gate opens regardless of window phase (9 MMs
    missed the phase in v2 and the gate stayed cold 12 us).
  - First chunks small ([1,1,2,4,4,4] k-tiles for block 0) so the first
    real matmuls start as early as the DMA ramp allows.
  - Epilogue per block: two fused scalar_tensor_tensor combines
    (lt = psM + 2^-11*psC per half), PE-transpose to [token, expert],
    DVE max8/max_index, ACT exp(accum_out), DVE reciprocal, ACT
    copy-scale normalize (balances DVE/ACT). One output DMA per block
    from a single u32 stage tile.
"""

import numpy as np

import concourse.mybir as mybir
import concourse.tile as tile
from concourse import bacc
from concourse.bass_utils import run_bass_kernel_spmd

f32 = mybir.dt.float32
f16 = mybir.dt.float16
u32 = mybir.dt.uint32
i32 = mybir.dt.int32

N_CORES = 8
B, S, H = 4, 4096, 2048
E = 64
TOP_K = 6
T_FULL = B * S              # 16384 tokens
T_CORE = T_FULL // N_CORES  # 2048 tokens per core
KT = H // 128               # 16 contraction tiles
NB = 4                      # token blocks per core
TB = T_CORE // NB           # 512 tokens per block
HB = TB // 2                # 256 tokens per column-group half
NTT = T_CORE // 128         # 16 token tiles per core
TPB = TB // 128             # 4 token tiles per block
OC = TOP_K + 8              # 14 staged u32 cols per token tile (6 w + 8 idx)
LSCALE = float(2.0 ** -11)
# k-tiles per DMA chunk, per block (1 k-tile = 256 KiB of stream data)
CHUNKS = [[1, 1, 2, 4, 4, 4]] + [[4, 4, 4, 4]] * (NB - 1)

_CACHE = {}


def _build():
    nc = bacc.Bacc("TRN2", target_bir_lowering=False, debug=False)
    # x stream layout [128, NB*KT*1024] fp16: col b*(KT*1024) + a*1024 +
    # half*512 + t, holding xh (half=0) / xl (half=1) for block b, k-tile a.
    x = nc.dram_tensor("x", [128, NB * KT * 1024], f16, kind="ExternalInput").ap()
    # weight pack [128, KT*128]: cols a*128+j = wh[j] (j<64) | wl[j-64]
    w2 = nc.dram_tensor("w2", [128, KT * 128], f16, kind="ExternalInput").ap()
    ident = nc.dram_tensor("ident", [E, E], f32, kind="ExternalInput").ap()
    out = nc.dram_tensor("out", [128, NTT * OC], u32, kind="ExternalOutput").ap()

    with tile.TileContext(nc) as tc:
        with (
            tc.tile_pool(name="persist", bufs=1) as persist,
            tc.tile_pool(name="work", bufs=4) as work,
            tc.tile_pool(name="psM", bufs=2, space="PSUM") as psMp,
            tc.tile_pool(name="psC", bufs=2, space="PSUM") as psCp,
            tc.tile_pool(name="psT", bufs=2, space="PSUM") as psTp,
            tc.tile_pool(name="psW", bufs=1, space="PSUM") as psWp,
        ):
            # ---- input DMAs, in stream order; first k-tiles' weights
            #      arrive before the bulk so the first matmuls start early
            w2_t = persist.tile([128, KT * 128], f16, tag="w2")
            nc.sync.dma_start(out=w2_t[:, 0:256], in_=w2[:, 0:256])
            nc.sync.dma_start(out=w2_t[:, 256:], in_=w2[:, 256:])
            id_t = persist.tile([E, E], f32, tag="ident")
            nc.sync.dma_start(out=id_t, in_=ident)
            # per-block chunk tiles; xat[(b, a)] -> (tile, col offset)
            xat = {}
            for b in range(NB):
                a0 = 0
                for ci, ksz in enumerate(CHUNKS[b]):
                    t = persist.tile([128, ksz * 1024], f16, tag=f"x{b}_{ci}")
                    src0 = (b * KT + a0) * 1024
                    nc.sync.dma_start(out=t, in_=x[:, src0 : src0 + ksz * 1024])
                    for j in range(ksz):
                        xat[(b, a0 + j)] = (t, j * 1024)
                    a0 += ksz

            stage = persist.tile([128, NTT * OC], u32, tag="stage")

            # ---- HAM warmup: ~7 us of junk matmuls covers a full activity
            #      window at any phase, opening the PE clock gate ----
            wm = persist.tile([128, 512], f16, tag="warm")
            nc.gpsimd.memset(wm, 0.25)
            wps = psWp.tile([128, 512], f32, tag="wps")
            for _ in range(16):
                nc.tensor.matmul(wps, wm[:, 0:128], wm, start=True, stop=True)

            def emit_k(psM, psC, b, a):
                ch, o = xat[(b, a)]
                wh = w2_t[:, a * 128 : a * 128 + 64]
                wl = w2_t[:, a * 128 + 64 : (a + 1) * 128]
                first, last = a == 0, a == KT - 1
                for g in range(2):
                    pr = slice(g * 64, (g + 1) * 64)
                    xh = ch[:, o + g * HB : o + (g + 1) * HB]
                    nc.tensor.matmul(psM[pr, :], wh, xh, start=first, stop=last)
                for g in range(2):
                    pr = slice(g * 64, (g + 1) * 64)
                    xh = ch[:, o + g * HB : o + (g + 1) * HB]
                    nc.tensor.matmul(psC[pr, :], wl, xh, start=first, stop=False)
                for g in range(2):
                    pr = slice(g * 64, (g + 1) * 64)
                    xl = ch[:, o + 512 + g * HB : o + 512 + (g + 1) * HB]
                    nc.tensor.matmul(psC[pr, :], wh, xl, start=False, stop=last)

            def emit_combine(psM, psC):
                # lt[:, g*256:+256] = psM[g] + 2^-11 * psC[g] per half.
                # (An op may read at most one PSUM input, so stage the
                # scaled correction through SBUF on the Scalar engine.)
                lt = work.tile([E, TB], f32, tag="lt")
                for g in range(2):
                    pr = slice(g * 64, (g + 1) * 64)
                    s = work.tile([E, HB], f32, tag="s")
                    nc.scalar.activation(
                        out=s, in_=psC[pr, :],
                        func=mybir.ActivationFunctionType.Copy, scale=LSCALE,
                    )
                    nc.vector.tensor_add(
                        lt[:, g * HB : (g + 1) * HB], s, psM[pr, :]
                    )
                return lt

            def emit_topk(b, lt):
                for tt in range(TPB):
                    t = b * TPB + tt
                    ps_t = psTp.tile([128, E], f32, tag="ps_t")
                    nc.tensor.transpose(ps_t, lt[:, tt * 128 : (tt + 1) * 128], id_t)
                    m8 = work.tile([128, 8], f32, tag="m8")
                    nc.vector.max(out=m8, in_=ps_t)
                    nc.vector.max_index(
                        stage[:, t * OC + TOP_K : (t + 1) * OC], m8, ps_t
                    )
                    expw = work.tile([128, TOP_K], f32, tag="expw")
                    ssum = work.tile([128, 1], f32, tag="ssum")
                    nc.scalar.activation(
                        out=expw,
                        in_=m8[:, 0:TOP_K],
                        func=mybir.ActivationFunctionType.Exp,
                        accum_out=ssum[:, 0:1],
                    )
                    rsum = work.tile([128, 1], f32, tag="rsum")
                    nc.vector.reciprocal(rsum, ssum)
                    nc.scalar.activation(
                        out=stage[:, t * OC : t * OC + TOP_K].bitcast(f32),
                        in_=expw,
                        func=mybir.ActivationFunctionType.Copy,
                        scale=rsum[:, 0:1],
                    )
                nc.sync.dma_start(
                    out=out[:, b * TPB * OC : (b + 1) * TPB * OC],
                    in_=stage[:, b * TPB * OC : (b + 1) * TPB * OC],
                )

            pending = None  # (b, lt) whose transposes/top-k are deferred
            for b in range(NB):
                psM = psMp.tile([128, HB], f32, tag="psM")
                psC = psCp.tile([128, HB], f32, tag="psC")
                for a in range(KT):
                    emit_k(psM, psC, b, a)
                    # previous block's PE transposes + top-k go a few
                    # k-tiles into this block so their ACT/DVE inputs are
                    # ready and the PE never stalls at the block boundary.
                    if a == 7 and pending is not None:
                        emit_topk(*pending)
                        pending = None
                pending = (b, emit_combine(psM, psC))
            emit_topk(*pending)

    nc.compile()
    return nc


def _get_nc():
    if "nc" not in _CACHE:
        _CACHE["nc"] = _build()
    return _CACHE["nc"]


def _split_fp16(arr32):
    """arr32 (fp32) -> (hi fp16, lo fp16) with arr32 ~= hi + 2^-11 * lo."""
    hi = arr32.astype(np.float16)
    lo = ((arr32 - hi.astype(np.float32)) * 2048.0).astype(np.float16)
    return hi, lo


def kernel(hidden_states: np.ndarray, weight: np.ndarray, **_run_kwargs):
    x = np.ascontiguousarray(hidden_states, dtype=np.float32).reshape(T_FULL, H)
    w = np.ascontiguousarray(weight, dtype=np.float32)

    w_hi, w_lo = _split_fp16(w)  # [E, H] fp16
    # [128, KT*128]: row p, col a*128+j  <-  (wh|wl)[j, a*128+p]
    wh_r = np.ascontiguousarray(w_hi.T).reshape(KT, 128, E)
    wl_r = np.ascontiguousarray(w_lo.T).reshape(KT, 128, E)
    w2p = np.ascontiguousarray(
        np.concatenate([wh_r, wl_r], axis=2).transpose(1, 0, 2).reshape(128, KT * 128)
    )
    ident = np.eye(E, dtype=np.float32)

    def pack_x(xT16_h, xT16_l):
        # [H, T_CORE] halves -> [128, NB*KT*1024] stream order
        vh = xT16_h.reshape(KT, 128, NB, TB)  # [a, p, b, t]
        vl = xT16_l.reshape(KT, 128, NB, TB)
        X = np.empty((128, NB, KT, 2, TB), dtype=np.float16)
        X[:, :, :, 0, :] = vh.transpose(1, 2, 0, 3)
        X[:, :, :, 1, :] = vl.transpose(1, 2, 0, 3)
        return np.ascontiguousarray(X.reshape(128, NB * KT * 1024))

    in_maps = []
    for c in range(N_CORES):
        shard = x[c * T_CORE : (c + 1) * T_CORE, :]  # [T_CORE, H]
        xT = np.ascontiguousarray(shard.T)  # [H, T_CORE] fp32
        xhs, xls = _split_fp16(xT)
        in_maps.append({"x": pack_x(xhs, xls), "w2": w2p, "ident": ident})

    nc = _get_nc()
    res = run_bass_kernel_spmd(
        nc, in_maps, core_ids=list(range(N_CORES)), **_run_kwargs
    )

    idx_parts = []
    w_parts = []
    for c in range(N_CORES):
        r = res.results[c]
        v = r["out"].reshape(128, NTT, OC).transpose(1, 0, 2)  # [tile, tok, col]
        idx = np.ascontiguousarray(v[:, :, TOP_K : TOP_K + TOP_K])
        wts = np.ascontiguousarray(v[:, :, 0:TOP_K]).view(np.uint32)
        idx_parts.append(
            idx.reshape(T_CORE, TOP_K).astype(np.int32, copy=False)
        )
        w_parts.append(wts.view(np.float32).reshape(T_CORE, TOP_K))

    topk_idx = np.concatenate(idx_parts, axis=0)
    topk_weight = np.concatenate(w_parts, axis=0)
    if "trace" in _run_kwargs:
        return (topk_idx, topk_weight), res
    return topk_idx, topk_weight


# revision 8
# speedup vs baseline: 1.0549x; 1.0135x over previous
"""MoE gate (top-6 routing) Trainium2 Bass kernel, v3.

Problem: hidden_states [4, 4096, 2048] f32, gate weight [64, 2048] f32.
  logits = x @ W.T            -> [16384, 64]
  topk_weight, topk_idx = top_k(logits, 6)
  topk_weight = softmax(topk_weight)
Returns (topk_idx int32 [16384, 6], topk_weight f32 [16384, 6]).

Sharding: data-parallel over tokens; 2048 tokens/core, weight replicated.

Precision (identical math to the verified baseline): x and w are split on
the host into fp16 halves, v = vh + 2^-11*vl, giving ~2^-22 relative
precision. logits = xh@wh + 2^-11*(xh@wl + xl@wh), bit-level top-6
agreement with the fp32 reference on the test inputs.

v3 structure (evidence-driven, from the v1/v2 traces):
  - Column-group concurrency: each 512-token block is split into two
    256-token halves assigned to PE column groups 0/1 (psum partitions
    0:64 / 64:128). Two matmuls in different column groups stream
    concurrently (~114 ns/MM measured for this pattern vs 379 ns for
    full-width), so the PE tracks the DMA stream instead of over-running
    it by 13 us. Stationaries are the 64-wide wh / wl k-tiles.
  - Block-major streaming: per block b and k-tile a the stream holds
    [xh | xl] (1024 cols); 6 N=256 matmuls per (b, a): xh@wh -> psM,
    xh@wl -> psC, xl@wh -> psC, each on both halves. psM/psC accumulate
    over the 16 k-tiles; per-block epilogue overlaps the next block's
    stream.
  - HAM warmup: 16 junk matmuls (~7 us cold) guarantee a full busy
    window so the PE clock-gate opens regardless of window phase (9 MMs
    missed the phase in v2 and the gate stayed cold 12 us).
  - First chunks small ([1,1,2,4,4,4] k-tiles for block 0) so the first
    real matmuls start as early as the DMA ramp allows.
  - Epilogue per block: two fused scalar_tensor_tensor combines
    (lt = psM + 2^-11*psC per half), PE-transpose to [token, expert],
    DVE max8/max_index, ACT exp(accum_out), DVE reciprocal, ACT
    copy-scale normalize (balances DVE/ACT). One output DMA per block
    from a single u32 stage tile.
"""

import numpy as np

import concourse.mybir as mybir
import concourse.tile as tile
from concourse import bacc
from concourse.bass_utils import run_bass_kernel_spmd

f32 = mybir.dt.float32
f16 = mybir.dt.float16
u32 = mybir.dt.uint32
i32 = mybir.dt.int32

N_CORES = 8
B, S, H = 4, 4096, 2048
E = 64
TOP_K = 6
T_FULL = B * S              # 16384 tokens
T_CORE = T_FULL // N_CORES  # 2048 tokens per core
KT = H // 128               # 16 contraction tiles
NB = 4                      # token blocks per core
TB = T_CORE // NB           # 512 tokens per block
HB = TB // 2                # 256 tokens per column-group half
NTT = T_CORE // 128         # 16 token tiles per core
TPB = TB // 128             # 4 token tiles per block
OC = TOP_K + 8              # 14 staged u32 cols per token tile (6 w + 8 idx)
LSCALE = float(2.0 ** -11)
# k-tiles per DMA chunk, per block (1 k-tile = 256 KiB of stream data).
# 512 KiB chunks: completion sems release PE work every ~1.2 us, keeping
# the PE's data-wait gaps well under the HAM idle-rethrottle window.
CHUNKS = [[1, 1] + [2] * 7] + [[2] * 8] * (NB - 1)

_CACHE = {}


def _build():
    nc = bacc.Bacc("TRN2", target_bir_lowering=False, debug=False)
    # x stream layout [128, NB*KT*1024] fp16: col b*(KT*1024) + a*1024 +
    # half*512 + t, holding xh (half=0) / xl (half=1) for block b, k-tile a.
    x = nc.dram_tensor("x", [128, NB * KT * 1024], f16, kind="ExternalInput").ap()
    # weight pack [128, KT*128]: cols a*128+j = wh[j] (j<64) | wl[j-64]
    w2 = nc.dram_tensor("w2", [128, KT * 128], f16, kind="ExternalInput").ap()
    ident = nc.dram_tensor("ident", [E, E], f32, kind="ExternalInput").ap()
    out = nc.dram_tensor("out", [128, NTT * OC], u32, kind="ExternalOutput").ap()

    with tile.TileContext(nc) as tc:
        with (
            tc.tile_pool(name="persist", bufs=1) as persist,
            tc.tile_pool(name="work", bufs=4) as work,
            tc.tile_pool(name="psM", bufs=2, space="PSUM") as psMp,
            tc.tile_pool(name="psC", bufs=2, space="PSUM") as psCp,
            tc.tile_pool(name="psT", bufs=2, space="PSUM") as psTp,
            tc.tile_pool(name="psW", bufs=1, space="PSUM") as psWp,
        ):
            # ---- input DMAs, in stream order; k-tiles 0-1's weights and
            #      the first two x chunks lead so the first matmuls can
            #      start as early as the DMA ramp allows
            w2_t = persist.tile([128, KT * 128], f16, tag="w2")
            id_t = persist.tile([E, E], f32, tag="ident")
            xat = {}  # (b, a) -> (tile, col offset)
            xtiles = []
            for b in range(NB):
                a0 = 0
                for ci, ksz in enumerate(CHUNKS[b]):
                    t = persist.tile([128, ksz * 1024], f16, tag=f"x{b}_{ci}")
                    src0 = (b * KT + a0) * 1024
                    xtiles.append((t, src0, ksz))
                    for j in range(ksz):
                        xat[(b, a0 + j)] = (t, j * 1024)
                    a0 += ksz
            nc.sync.dma_start(out=w2_t[:, 0:256], in_=w2[:, 0:256])
            for t, src0, ksz in xtiles[:2]:
                nc.sync.dma_start(out=t, in_=x[:, src0 : src0 + ksz * 1024])
            nc.sync.dma_start(out=w2_t[:, 256:], in_=w2[:, 256:])
            nc.sync.dma_start(out=id_t, in_=ident)
            for t, src0, ksz in xtiles[2:]:
                nc.sync.dma_start(out=t, in_=x[:, src0 : src0 + ksz * 1024])

            stage = persist.tile([128, NTT * OC], u32, tag="stage")

            # ---- HAM warmup: ~7 us of junk matmuls covers a full activity
            #      window at any phase, opening the PE clock gate ----
            wm = persist.tile([128, 512], f16, tag="warm")
            nc.gpsimd.memset(wm, 0.25)
            wps = psWp.tile([128, 512], f32, tag="wps")
            for _ in range(12):
                nc.tensor.matmul(wps, wm[:, 0:128], wm, start=True, stop=True)

            def emit_k(psM, psC, b, a):
                ch, o = xat[(b, a)]
                wh = w2_t[:, a * 128 : a * 128 + 64]
                wl = w2_t[:, a * 128 + 64 : (a + 1) * 128]
                first, last = a == 0, a == KT - 1
                for g in range(2):
                    pr = slice(g * 64, (g + 1) * 64)
                    xh = ch[:, o + g * HB : o + (g + 1) * HB]
                    nc.tensor.matmul(psM[pr, :], wh, xh, start=first, stop=last)
                for g in range(2):
                    pr = slice(g * 64, (g + 1) * 64)
                    xh = ch[:, o + g * HB : o + (g + 1) * HB]
                    nc.tensor.matmul(psC[pr, :], wl, xh, start=first, stop=False)
                for g in range(2):
                    pr = slice(g * 64, (g + 1) * 64)
                    xl = ch[:, o + 512 + g * HB : o + 512 + (g + 1) * HB]
                    nc.tensor.matmul(psC[pr, :], wh, xl, start=False, stop=last)

            def emit_combine(psM, psC):
                # lt[:, g*256:+256] = psM[g] + 2^-11 * psC[g] per half.
                # (An op may read at most one PSUM input, so stage the
                # scaled correction through SBUF on the Scalar engine.)
                lt = work.tile([E, TB], f32, tag="lt")
                for g in range(2):
                    pr = slice(g * 64, (g + 1) * 64)
                    s = work.tile([E, HB], f32, tag="s")
                    nc.scalar.activation(
                        out=s, in_=psC[pr, :],
                        func=mybir.ActivationFunctionType.Copy, scale=LSCALE,
                    )
                    nc.vector.tensor_add(
                        lt[:, g * HB : (g + 1) * HB], s, psM[pr, :]
                    )
                return lt

            def emit_topk(b, lt):
                for tt in range(TPB):
                    t = b * TPB + tt
                    ps_t = psTp.tile([128, E], f32, tag="ps_t")
                    nc.tensor.transpose(ps_t, lt[:, tt * 128 : (tt + 1) * 128], id_t)
                    m8 = work.tile([128, 8], f32, tag="m8")
                    nc.vector.max(out=m8, in_=ps_t)
                    nc.vector.max_index(
                        stage[:, t * OC + TOP_K : (t + 1) * OC], m8, ps_t
                    )
                    expw = work.tile([128, TOP_K], f32, tag="expw")
                    ssum = work.tile([128, 1], f32, tag="ssum")
                    nc.scalar.activation(
                        out=expw,
                        in_=m8[:, 0:TOP_K],
                        func=mybir.ActivationFunctionType.Exp,
                        accum_out=ssum[:, 0:1],
                    )
                    rsum = work.tile([128, 1], f32, tag="rsum")
                    nc.vector.reciprocal(rsum, ssum)
                    # alternate the normalize between ACT and DVE so
                    # neither engine serializes the 4-tile chain
                    if tt % 2 == 0:
                        nc.scalar.activation(
                            out=stage[:, t * OC : t * OC + TOP_K].bitcast(f32),
                            in_=expw,
                            func=mybir.ActivationFunctionType.Copy,
                            scale=rsum[:, 0:1],
                        )
                    else:
                        nc.vector.tensor_scalar_mul(
                            stage[:, t * OC : t * OC + TOP_K].bitcast(f32),
                            expw,
                            rsum[:, 0:1],
                        )
                nc.sync.dma_start(
                    out=out[:, b * TPB * OC : (b + 1) * TPB * OC],
                    in_=stage[:, b * TPB * OC : (b + 1) * TPB * OC],
                )

            pending = None  # (b, lt) whose transposes/top-k are deferred
            for b in range(NB):
                psM = psMp.tile([128, HB], f32, tag="psM")
                psC = psCp.tile([128, HB], f32, tag="psC")
                for a in range(KT):
                    emit_k(psM, psC, b, a)
                    # previous block's PE transposes + top-k go a few
                    # k-tiles into this block so their ACT/DVE inputs are
                    # ready and the PE never stalls at the block boundary.
                    if a == 7 and pending is not None:
                        emit_topk(*pending)
                        pending = None
                pending = (b, emit_combine(psM, psC))
            emit_topk(*pending)

    nc.compile()
    return nc


def _get_nc():
    if "nc" not in _CACHE:
        _CACHE["nc"] = _build()
    return _CACHE["nc"]


def _split_fp16(arr32):
    """arr32 (fp32) -> (hi fp16, lo fp16) with arr32 ~= hi + 2^-11 * lo."""
    hi = arr32.astype(np.float16)
    lo = ((arr32 - hi.astype(np.float32)) * 2048.0).astype(np.float16)
    return hi, lo


def kernel(hidden_states: np.ndarray, weight: np.ndarray, **_run_kwargs):
    x = np.ascontiguousarray(hidden_states, dtype=np.float32).reshape(T_FULL, H)
    w = np.ascontiguousarray(weight, dtype=np.float32)

    w_hi, w_lo = _split_fp16(w)  # [E, H] fp16
    # [128, KT*128]: row p, col a*128+j  <-  (wh|wl)[j, a*128+p]
    wh_r = np.ascontiguousarray(w_hi.T).reshape(KT, 128, E)
    wl_r = np.ascontiguousarray(w_lo.T).reshape(KT, 128, E)
    w2p = np.ascontiguousarray(
        np.concatenate([wh_r, wl_r], axis=2).transpose(1, 0, 2).reshape(128, KT * 128)
    )
    ident = np.eye(E, dtype=np.float32)

    def pack_x(xT16_h, xT16_l):
        # [H, T_CORE] halves -> [128, NB*KT*1024] stream order
        vh = xT16_h.reshape(KT, 128, NB, TB)  # [a, p, b, t]
        vl = xT16_l.reshape(KT, 128, NB, TB)
        X = np.empty((128, NB, KT, 2, TB), dtype=np.float16)
        X[:, :, :, 0, :] = vh.transpose(1, 2, 0, 3)
        X[:, :, :, 1, :] = vl.transpose(1, 2, 0, 3)
        return np.ascontiguousarray(X.reshape(128, NB * KT * 1024))

    in_maps = []
    for c in range(N_CORES):
        shard = x[c * T_CORE : (c + 1) * T_CORE, :]  # [T_CORE, H]
        xT = np.ascontiguousarray(shard.T)  # [H, T_CORE] fp32
        xhs, xls = _split_fp16(xT)
        in_maps.append({"x": pack_x(xhs, xls), "w2": w2p, "ident": ident})

    nc = _get_nc()
    res = run_bass_kernel_spmd(
        nc, in_maps, core_ids=list(range(N_CORES)), **_run_kwargs
    )

    idx_parts = []
    w_parts = []
    for c in range(N_CORES):
        r = res.results[c]
        v = r["out"].reshape(128, NTT, OC).transpose(1, 0, 2)  # [tile, tok, col]
        idx = np.ascontiguousarray(v[:, :, TOP_K : TOP_K + TOP_K])
        wts = np.ascontiguousarray(v[:, :, 0:TOP_K]).view(np.uint32)
        idx_parts.append(
            idx.reshape(T_CORE, TOP_K).astype(np.int32, copy=False)
        )
        w_parts.append(wts.view(np.float32).reshape(T_CORE, TOP_K))

    topk_idx = np.concatenate(idx_parts, axis=0)
    topk_weight = np.concatenate(w_parts, axis=0)
    if "trace" in _run_kwargs:
        return (topk_idx, topk_weight), res
    return topk_idx, topk_weight
